# revision 29
# baseline (speedup 1.0000x reference)
"""Trainium2 Bass kernel for BaseAttention (B=4, S=2048, H=16 heads x 64).

Sharding: 8 cores = 4 batches x 2 head-groups (8 heads / 512 dims each).
Each core computes q/k/v projections for its head group on its batch,
flash-style causal attention (scores never leave the chip), and a partial
o-projection over its 512 head dims. The host sums the two partial outputs
per batch.

All matmul inputs are bf16 (f32 PSUM accumulation). bf16 keeps the PE at
1 cycle/row like f32r but enables the compiler's fast-weight-load path
(FWL needs dtype != fp32 and 128 stationary columns), hiding LDWEIGHTS
behind the moving stream, and halves DMA/SBUF/DVE traffic.

V layout per (seq-chunk, head-pair): a 194-column block
  [v_even(0:64) | ones(64:130) | v_odd(130:194)]
so the PV matmul for the even head uses block[0:128] (attn rows 0:64,
denominator row 64) and the odd head uses block[66:194] (attn rows 64:128,
denominator row 0 -- PE row-offset placements other than 0/64 misbehave
on HW, so both denominators sit at known-good rows). Both give 128 stationary columns (FWL) and write
attnT at partition offsets 0/64 without any cross-partition copy, so the
o-projection reads attnT straight from SBUF (no DRAM spill).
"""

import numpy as np

B = 4
S = 2048
HIDDEN = 1024
NH = 16
DH = 64
HG = 2                  # head groups (cores per batch)
DG = HIDDEN // HG       # 512 dims per group (8 heads)
NCORES = B * HG
SCALE = DH ** -0.5

P = 128
KC = HIDDEN // P        # 8 contraction chunks for projections
NQ = S // 512           # 4 query chunks of 512
SM = S // P             # 16 seq chunks of 128
MCH = DG // P           # 4 chunks of 128 over the group's 512 dims
NHG = NH // HG          # 8 heads per core
NJ = NHG // 2           # 4 head pairs per core
VBS = 194               # v block stride (v_e | ones-fill | v_o)
VO = 130                # v_odd offset within block
ODD = 66                # odd-head lhsT window offset (ones -> row 0)

_CACHE = {}


def _emit(nc, tc, tens):
    import concourse.mybir as mybir
    import concourse.bass as bass
    from collections import deque
    from contextlib import ExitStack

    f32 = mybir.dt.float32
    f32r = mybir.dt.float32r
    bf16 = mybir.dt.bfloat16
    Exp = mybir.ActivationFunctionType.Exp
    ActCopy = mybir.ActivationFunctionType.Copy
    ActRecip = mybir.ActivationFunctionType.Reciprocal
    mult = mybir.AluOpType.mult
    ds = bass.ds

    xT = tens["xT"].ap().rearrange("(kc p) s -> p kc s", p=P)
    wqT = tens["wqT"].ap().rearrange("(kc p) d -> p kc d", p=P)
    wkT = tens["wkT"].ap().rearrange("(kc p) d -> p kc d", p=P)
    wvT = tens["wvT"].ap().rearrange("(kc p) d -> p kc d", p=P)
    woT = tens["woT"].ap().rearrange("(ic p) j -> p ic j", p=P)
    masks = tens["masks"].ap()
    out = tens["out"].ap().rearrange("(sm p) j -> p sm j", p=P)

    with ExitStack() as ctx:
        persist = ctx.enter_context(tc.tile_pool(name="persist", bufs=1))
        ps_sc = ctx.enter_context(tc.tile_pool(name="ps_sc", bufs=2, space="PSUM"))
        ps_pj = ctx.enter_context(tc.tile_pool(name="ps_pj", bufs=2, space="PSUM"))
        ps_at = ctx.enter_context(tc.tile_pool(name="ps_at", bufs=2, space="PSUM"))
        pstage = ctx.enter_context(tc.tile_pool(name="pstage", bufs=2))
        ptp = ctx.enter_context(tc.tile_pool(name="pt", bufs=6))
        recp = ctx.enter_context(tc.tile_pool(name="rec", bufs=2))
        ostp = ctx.enter_context(tc.tile_pool(name="ost", bufs=3))
        qpool = ctx.enter_context(tc.tile_pool(name="qp", bufs=2))

        kTz = persist.tile([P, NHG, S], bf16)            # per-head k^T, zero-
        #   padded to 128 rows: even heads live in rows 0:64 (rows 64:128
        #   zero), odd heads in rows 64:128 -- every score matmul is then a
        #   full [128,128]-weight config like PV/proj (uniform PE shape)
        v_sb = persist.tile([P, SM, NJ, VBS], bf16)      # v blocks + ones cols
        attnT_sb = persist.tile([P, MCH, S], bf16)       # normalized attn^T
        wq_sb = persist.tile([P, KC, DG], bf16)
        wk_sb = persist.tile([P, KC, DG], bf16)
        wv_sb = persist.tile([P, KC, DG], bf16)
        wo_sb = persist.tile([P, MCH, HIDDEN], bf16)
        mask_sb = persist.tile([P, P], bf16)   # triangular allow(q>=k)

        ones_f32 = persist.tile([P, 1], f32)
        nc.vector.memset(ones_f32[:], 1.0)  # f32r/bf16 memset fails ISA checks

        nc.vector.tensor_copy(
            out=v_sb[:, :, :, DH:VO],
            in_=ones_f32[:, 0:1].to_broadcast([P, SM, NJ, VO - DH]),
        )

        # Denominator broadcast helpers: one-hot selector rows (64 for even
        # heads, 0 for odd) times a zeroed staging buffer whose only written
        # rows are 64/0. Keeps every matmul a standard 128x128-weight config;
        # sub-64-row weight tiles write garbage to out partitions 64:128 on HW.
        zeros_f32 = persist.tile([P, 1], f32)
        nc.vector.memset(zeros_f32[:], 0.0)
        sel_f32 = persist.tile([P, 1, P], f32)
        nc.vector.memset(sel_f32[:], 0.0)
        nc.vector.memset(sel_f32[DH:DH + 1, 0, 0:DH], 1.0)
        nc.vector.memset(sel_f32[0:1, 0, DH:P], 1.0)
        sel_sb = persist.tile([P, 1, P], bf16)
        nc.vector.tensor_copy(out=sel_sb[:], in_=sel_f32[:])
        rec_sb = persist.tile([P, 512], bf16)
        nc.vector.tensor_copy(out=rec_sb[:], in_=zeros_f32[:, 0:1].to_broadcast([P, 512]))
        nc.vector.tensor_copy(out=kTz[:], in_=zeros_f32[:, 0:1].to_broadcast([P, NHG, S]))

        xts = {}
        qsbs = {}
        if "dbg_acc" in tens:
            dbg_acc_sb = persist.tile([P, 2, 512], f32, name="dbgacc")
            dbg_bc_sb = persist.tile([P, 2, 512], f32, name="dbgbc")

        def proj_closures(n):
            """q/k/v projection work for seq chunk n: 13 closures."""
            cls = []

            def load_xt(n=n):
                xt = pstage.tile([P, KC, 512], bf16, tag="xt")
                nc.sync.dma_start(xt[:], xT[:, :, ds(n * 512, 512)])
                xts[n] = xt
                qsbs[n] = qpool.tile([P, MCH, 512], bf16, tag="qsb", name=f"qsb_{n}")
            cls.append(load_xt)

            for w_sb, dst_name in ((wq_sb, "q"), (wk_sb, "k")):
                for m in range(MCH):
                    def qk_group(n=n, w_sb=w_sb, dst_name=dst_name, m=m):
                        xt = xts[n]
                        ps = ps_pj.tile([P, 512], f32, tag="pj")
                        for kc in range(KC):
                            nc.tensor.matmul(
                                ps[:],
                                w_sb[:, kc, ds(m * P, P)],
                                xt[:, kc, :],
                                start=(kc == 0), stop=(kc == KC - 1),
                            )
                        if dst_name == "k":
                            nc.vector.tensor_copy(
                                out=kTz[0:DH, 2 * m, ds(n * 512, 512)],
                                in_=ps[0:DH, :],
                            )
                            nc.vector.tensor_copy(
                                out=kTz[DH:P, 2 * m + 1, ds(n * 512, 512)],
                                in_=ps[DH:P, :],
                            )
                        else:
                            nc.vector.tensor_copy(
                                out=qsbs[n][:, m, :], in_=ps[:]
                            )
                    cls.append(qk_group)

            for sm in range(4 * n, 4 * n + 4):
                def v_group(n=n, sm=sm):
                    xt = xts[n]
                    ps = ps_pj.tile([P, 512], f32, tag="pj")
                    for kc in range(KC):
                        nc.tensor.matmul(
                            ps[:],
                            xt[:, kc, ds((sm - 4 * n) * P, P)],
                            wv_sb[:, kc, :],
                            start=(kc == 0), stop=(kc == KC - 1),
                        )
                    pv = ps[:].rearrange("p (j d) -> p j d", j=NJ)
                    nc.vector.tensor_copy(
                        out=v_sb[:, sm, :, 0:DH], in_=pv[:, :, 0:DH]
                    )
                    nc.vector.tensor_copy(
                        out=v_sb[:, sm, :, VO:VO + DH], in_=pv[:, :, DH:2 * DH]
                    )
                cls.append(v_group)
            return cls

        def oproj_closures(n, act_cast=False):
            """o-projection for seq chunk n: 8 closures (one per sm half)."""
            cls = []
            for sm in range(4 * n, 4 * n + 4):
                for j2 in range(2):
                    def o_group(sm=sm, j2=j2, act_cast=act_cast):
                        ps = ps_pj.tile([P, 512], f32, tag="pj")
                        for ic in range(MCH):
                            nc.tensor.matmul(
                                ps[:],
                                attnT_sb[:, ic, ds(sm * P, P)],
                                wo_sb[:, ic, ds(j2 * 512, 512)],
                                start=(ic == 0), stop=(ic == MCH - 1),
                            )
                        ost = ostp.tile([P, 512], bf16, tag="ost")
                        if act_cast:
                            nc.scalar.activation(ost[:], ps[:], ActCopy)
                        else:
                            nc.vector.tensor_copy(out=ost[:], in_=ps[:])
                        nc.sync.dma_start(out[:, sm, ds(j2 * 512, 512)], ost[:])
                    cls.append(o_group)
            return cls

        # startup: xt(0)+wq first so the PE starts within a few us; the
        # remaining weight DMAs stream behind the first matmul groups.
        csl = ds(0, DG // 4)
        nc.sync.dma_start(wq_sb[:, :, csl], wqT[:, :, csl])
        p0 = proj_closures(0)
        p0[0]()                                   # xt(0) + qsb alloc
        for c4 in range(1, 4):
            csl = ds(c4 * (DG // 4), DG // 4)
            nc.sync.dma_start(wq_sb[:, :, csl], wqT[:, :, csl])
        for c in p0[1:5]:                         # q groups
            c()
        for c4 in range(4):
            csl = ds(c4 * (DG // 4), DG // 4)
            nc.sync.dma_start(wk_sb[:, :, csl], wkT[:, :, csl])
        for c in p0[5:9]:                         # k groups
            c()
        for c4 in range(4):
            csl = ds(c4 * (DG // 4), DG // 4)
            nc.sync.dma_start(wv_sb[:, :, csl], wvT[:, :, csl])
        nc.sync.dma_start(mask_sb[:], masks)
        for c4 in range(4):
            csl = ds(c4 * (HIDDEN // 4), HIDDEN // 4)
            nc.sync.dma_start(wo_sb[:, :, csl], woT[:, :, csl])

        filler = deque()
        filler.extend(p0[9:])                     # n=0 v groups: paced into
        #                                           n=0's attention phase
        pending = []  # deferred normalization closures

        def flush_pending():
            for c in pending:
                c()
            pending.clear()

        def norm_closure(n, j, acc):
            # both heads of pair j: even denom -> rec row 64, odd -> row 0.
            # One selector matmul broadcasts them to rows 0:64 / 64:128 of a
            # single bc tile, one reciprocal serves both TTs.
            qsl = ds(n * 512, 512)
            # denom row copies + reciprocal run on the Scalar engine: at the
            # j boundary Act is idle while DVE has a queue, and the next
            # pair's first PV waits on this chain (acc buffer reuse)
            with nc.allow_low_precision(reason="denom row stage"):
                nc.vector.tensor_copy(out=rec_sb[DH:DH + 1, :],
                                      in_=acc[0][DH:DH + 1, :])
                nc.vector.tensor_copy(out=rec_sb[0:1, :],
                                      in_=acc[1][0:1, :])

            def finish():
                bc_full = ps_pj.tile([P, 512], f32, tag="pj", name=f"bc_{n}_{j}")
                bc = bc_full[:]
                nc.tensor.matmul(bc, sel_sb[:, 0, :], rec_sb[:, :],
                                 start=True, stop=True)
                # full-width rcp: custom DVE ops silently no-op at partition
                # bases other than 0 on HW
                rcp = recp.tile([P, 512], f32, tag="rcp")
                nc.vector.reciprocal_approx_fast(rcp[:], bc)
                if "dbg_acc" in tens and n == 3 and j == 0:
                    for e in range(2):
                        nc.vector.tensor_copy(out=dbg_acc_sb[:, e, :], in_=acc[e][:])
                    nc.vector.tensor_copy(out=dbg_bc_sb[:, 0, :], in_=bc)
                    nc.sync.dma_start(tens["dbg_acc"].ap(), dbg_acc_sb[:])
                    nc.sync.dma_start(tens["dbg_bc"].ap(), dbg_bc_sb[:])
                nc.vector.tensor_tensor(
                    attnT_sb[0:DH, j, qsl], acc[0][0:DH, :],
                    rcp[0:DH, :], mult,
                )
                nc.vector.tensor_tensor(
                    attnT_sb[DH:P, j, qsl], acc[1][DH:P, :],
                    rcp[DH:P, :], mult,
                )
            return finish

        for n in range(NQ):
            if n + 1 < NQ:
                pc = proj_closures(n + 1)
                pc[0]()                       # start xt(n+1) DMA immediately
                filler.extend(pc[1:])
            if n == 2:
                filler.extend(oproj_closures(0))
            elif n == 3:
                filler.extend(oproj_closures(1))
                filler.extend(oproj_closures(2))
            npairs = 2 * (n + 1)
            total_pairs = NJ * npairs
            pace_num = len(filler)
            pace_acc = 0
            for j in range(NJ):
                acc = [
                    ps_at.tile([P, 512], f32, tag="acc",
                               name=f"acc_{n}_{j}_{e}")
                    for e in range(2)
                ]
                pvq = []  # deferred PV matmuls (consumed 3 pairs later)

                def emit_pv():
                    tp, e, u, pt = pvq.pop(0)
                    m = 2 * tp + u
                    off = max(0, P * (m - 4 * n))   # causal suffix of diag blocks
                    voff = 0 if e == 0 else ODD
                    nc.tensor.matmul(
                        acc[e][:, ds(off, 512 - off)],
                        v_sb[:, m, j, ds(voff, P)],
                        pt[:, ds(u * 512 + off, 512 - off)],
                        start=(tp == 0 and u == 0),
                        stop=(tp == npairs - 1 and u == 1),
                    )

                for t in range(npairs):
                    if pending:
                        flush_pending()
                    # pump interleaved proj/o-proj work in bursts of >=2
                    # groups: a dense >3.4us PE stretch lets the HAM clock
                    # gate open (scattered 1-group pumps never do)
                    pace_acc += pace_num
                    while pace_acc >= total_pairs and filler:
                        filler.popleft()()
                        pace_acc -= total_pairs
                    diag = t >= 2 * n
                    new_pvq = []
                    pss = []
                    # batch the 4 score matmuls (same K=64 weight shape),
                    # then the due PVs (K=128): fewer weight-shape/bank
                    # switches per t than a fine interleave
                    for e in range(2):          # head pair member
                        ps = ps_sc.tile([P, 1024], f32, tag="sc")
                        pss.append(ps)
                        for u in range(2):      # m-pair member
                            m = 2 * t + u
                            off = max(0, P * (m - 4 * n))
                            nc.tensor.matmul(
                                ps[:, ds(u * 512 + off, 512 - off)],
                                kTz[:, 2 * j + e, ds(m * P, P)],
                                qsbs[n][:, j, ds(off, 512 - off)],
                                start=True, stop=True,
                            )
                    for e in range(2):
                        ps = pss[e]
                        pt = ptp.tile([P, 1024], bf16, tag="pt")
                        if diag:
                            off0 = P * (2 * t - 4 * n)
                            # per-block exps over the causal suffix only (the
                            # psum ownership checker forbids touching the
                            # unwritten gap between the two blocks)
                            for u in range(2):
                                sl = ds(u * 512 + off0 + u * P,
                                        512 - off0 - u * P)
                                nc.scalar.activation(pt[:, sl], ps[:, sl],
                                                     Exp, scale=SCALE)
                            # triangular strip mask per m-block
                            for u in range(2):
                                soff = u * 512 + off0 + u * P
                                nc.vector.tensor_tensor(
                                    pt[:, ds(soff, P)], pt[:, ds(soff, P)],
                                    mask_sb[:], mult,
                                )
                        else:
                            nc.scalar.activation(pt[:], ps[:], Exp, scale=SCALE)
                        new_pvq.extend((t, e, u, pt) for u in range(2))
                    while pvq and pvq[0][0] <= t - 3:
                        emit_pv()
                    pvq.extend(new_pvq)
                while pvq:
                    emit_pv()
                pending.append(norm_closure(n, j, acc))
            while filler:
                filler.popleft()()
        flush_pending()
        for c in oproj_closures(NQ - 1, act_cast=True):
            c()
        if "dbg_att" in tens:
            nc.sync.dma_start(tens["dbg_att"].ap(), attnT_sb[:])
            nc.sync.dma_start(tens["dbg_v"].ap(), v_sb[:])
            nc.sync.dma_start(tens["dbg_kt"].ap(), kTz[:])
            nc.sync.dma_start(tens["dbg_q"].ap(), qsbs[NQ - 1][:])


def _build():
    import concourse.mybir as mybir
    import concourse.tile as tile
    from concourse import bacc

    bf16 = mybir.dt.bfloat16
    nc = bacc.Bacc("TRN2", target_bir_lowering=False, debug=False,
                   num_devices=NCORES)
    tens = {
        "xT": nc.dram_tensor("xT", [HIDDEN, S], bf16, kind="ExternalInput"),
        "wqT": nc.dram_tensor("wqT", [HIDDEN, DG], bf16, kind="ExternalInput"),
        "wkT": nc.dram_tensor("wkT", [HIDDEN, DG], bf16, kind="ExternalInput"),
        "wvT": nc.dram_tensor("wvT", [HIDDEN, DG], bf16, kind="ExternalInput"),
        "woT": nc.dram_tensor("woT", [DG, HIDDEN], bf16, kind="ExternalInput"),
        "masks": nc.dram_tensor("masks", [P, P], bf16, kind="ExternalInput"),
        "out": nc.dram_tensor("out", [S, HIDDEN], bf16, kind="ExternalOutput"),
    }
    import os
    if os.environ.get("KDBG") == "1":
        tens["dbg_att"] = nc.dram_tensor("dbg_att", [P, MCH, S], bf16, kind="ExternalOutput")
        tens["dbg_v"] = nc.dram_tensor("dbg_v", [P, SM, NJ, VBS], bf16, kind="ExternalOutput")
        tens["dbg_kt"] = nc.dram_tensor("dbg_kt", [P, MCH, S], bf16, kind="ExternalOutput")
        tens["dbg_q"] = nc.dram_tensor("dbg_q", [P, MCH, 512], bf16, kind="ExternalOutput")
        tens["dbg_acc"] = nc.dram_tensor("dbg_acc", [P, 2, 512], mybir.dt.float32, kind="ExternalOutput")
        tens["dbg_bc"] = nc.dram_tensor("dbg_bc", [P, 2, 512], mybir.dt.float32, kind="ExternalOutput")
    with tile.TileContext(nc) as tc:
        _emit(nc, tc, tens)
    nc.compile()
    return nc


def get_program():
    if "nc" not in _CACHE:
        _CACHE["nc"] = _build()
    return _CACHE["nc"]


def make_in_maps(hidden_states, attention_mask, wq, wk, wv, wo):
    """Build the per-core input maps (host-side sharding)."""
    import ml_dtypes
    bf = ml_dtypes.bfloat16
    hidden_states = np.asarray(hidden_states, dtype=np.float32)
    attention_mask = np.asarray(attention_mask, dtype=np.float32)
    wq = np.asarray(wq, dtype=np.float32)
    wk = np.asarray(wk, dtype=np.float32)
    wv = np.asarray(wv, dtype=np.float32)
    wo = np.asarray(wo, dtype=np.float32)

    # Single triangular strip mask for the 128-wide diagonal blocks of
    # scores^T, derived from the provided additive mask (0 = attend,
    # big negative = blocked): mask[kk, qq] = allow(q = qq, k = kk).
    am = attention_mask[0, 0]
    mask_np = (am[0:P, 0:P] == 0.0).T.astype(np.float32).astype(bf)

    in_maps = []
    for c in range(NCORES):
        b, g = divmod(c, HG)
        rows = slice(g * DG, (g + 1) * DG)
        in_maps.append({
            "xT": np.ascontiguousarray(hidden_states[b].T).astype(bf),
            "wqT": np.ascontiguousarray(wq[rows, :].T).astype(bf),
            "wkT": np.ascontiguousarray(wk[rows, :].T).astype(bf),
            "wvT": np.ascontiguousarray(wv[rows, :].T).astype(bf),
            "woT": np.ascontiguousarray(wo[:, rows].T).astype(bf),
            "masks": mask_np,
        })
    return in_maps


def combine_outputs(results):
    out = np.empty((B, S, HIDDEN), dtype=np.float32)
    for b in range(B):
        out[b] = (results[HG * b]["out"].astype(np.float32)
                  + results[HG * b + 1]["out"].astype(np.float32))
    return out


def kernel(hidden_states, attention_mask, wq, wk, wv, wo):
    from concourse.bass_utils import run_bass_kernel_spmd

    nc = get_program()
    in_maps = make_in_maps(hidden_states, attention_mask, wq, wk, wv, wo)
    res = run_bass_kernel_spmd(nc, in_maps, list(range(NCORES)))
    return combine_outputs(res.results)
